# revision 53
# baseline (speedup 1.0000x reference)
"""DA-RNN (dual-stage attention RNN) Trainium2 Bass kernel.

Problem: nn_Darnn_14774687498323.  B=2048, T=64, N=256, M=P=256.
Sharding: pure data parallel, batch split 8 ways (BC=256 per core), weights
replicated.  Full inputs in, full outputs out.

Per-core layout conventions
---------------------------
b       : per-core batch index, 0..255.  b-half bh = b // 128.
pair i  : i = b >> 1 (128 pairs); parity = b & 1.
p'      : "pair partition" = s + 64*parity packs the 64 attention positions
          of two consecutive batch rows into 128 partitions.

Encoder attention:  e[b,n] = sum_s ve[s] * tanh(x[b,s] + y[b,n,s]) with
y = einsum('btn,st->bns').  y is precomputed to DRAM in the [p', i, n]
layout and streamed each step; the x-add runs as per-partition-scalar DVE
adds, tanh as bulk in-place ACT, and the ve-sum as one tiny PE matmul per
pair with a block-diagonal 2-column ve (out = e rows [2i, 2i+1]).

Decoder attention: l[b,t] = sum_m vd[m] * tanh(x1[b,m] + y1[b,t,m]).
y1 is built incrementally during the encoder in the [m-half][mp, t, b]
layout (b innermost) and streamed each decoder step; the x1-add runs as
bf16 tensor_tensor with an outer-dim-0 broadcast AP, the vd-sum as clean
K=128 matmuls into an l-psum of shape [32, 512] = [flat(t,b)/512, .].
Softmax-over-t sums come from a ones-vector matmul + strided DVE reduce;
unnormalized exp(l) is scattered into the block-diag buffer for the c_t
pair-matmuls and normalization is folded into the c_t psum eviction.

sigmoid(x) = 0.5*tanh(x/2) + 0.5 everywhere so the ACT table set never
leaves exp_and_others (tanh + exp), avoiding ~2.7us table reloads.
"""

import os
import numpy as np
import ml_dtypes
from contextlib import ExitStack

import concourse.bass as bass
import concourse.tile as tile
from concourse import bacc, mybir
from concourse.bass_utils import run_bass_kernel_spmd

F32 = mybir.dt.float32
BF = mybir.dt.bfloat16
bf16 = ml_dtypes.bfloat16
AF = mybir.ActivationFunctionType
OP = mybir.AluOpType

T, N, M, B = 64, 256, 256, 2048
NCORES = 8
BC = B // NCORES          # 256 batch rows per core
NP = BC // 2              # 128 pairs

DEBUG = bool(int(os.environ.get("DARNN_DEBUG", "0")))

_CACHED_NC = None


def _bcast(ap, n, axis):
    """Insert a step-0 dim of size n at free-dim position `axis` (1-based in ap list)."""
    new = list(ap.ap)
    new.insert(axis, [0, n])
    return bass.AP(tensor=ap.tensor, offset=ap.offset, ap=new)


def build_nc():
    nc = bacc.Bacc("TRN2", target_bir_lowering=False, debug=False,
                   num_devices=NCORES)

    dram = {}

    def din(name, shape, dt):
        dram[name] = nc.dram_tensor(name, shape, dt, kind="ExternalInput").ap()
        return dram[name]

    X = din("X", [BC, T, N], F32)
    UeT = din("UeT", [T, T], F32)
    vzsel = din("vzsel", [128, 32, 64], BF)
    WeT = din("WeT", [512, T], BF)
    Web128 = din("Web128", [128, 1], F32)
    encG = din("encG", [512, 1024], BF)
    encGb = din("encGb", [1, 1024], BF)
    UdT = din("UdT", [256, 256], BF)
    vdsel = din("vdsel", [128, 2, 32, 32], BF)
    WdT = din("WdT", [512, 256], BF)
    WdTb = din("WdTb", [1, 256], BF)
    decG = din("decG", [256, 1024], BF)
    decG2 = din("decG2", [2, 1024], BF)
    wt2 = din("wt2", [256, 1], BF)
    WyT = din("WyT", [512, 256], F32)
    WyTb = din("WyTb", [1, 256], F32)
    vyR = din("vyR", [128, 256], F32)
    vyb128 = din("vyb128", [128, 1], F32)
    identD = din("identD", [128, 128], BF)
    identF = din("identF", [128, 128], F32)

    OUT = nc.dram_tensor("OUT", [BC, 1], F32, kind="ExternalOutput").ap()
    dbg = {}
    if DEBUG:
        for nm, shp in [("D_xs0", [128, 128]), ("D_e0", [128, 256]),
                        ("D_wx0", [128, 2, 256]), ("D_h0", [128, 2, 256]),
                        ("D_hf", [128, 2, 256]), ("D_cf", [128, 2, 256]),
                        ("D_expl0", [32, 512]), ("D_ct0", [128, 2, 256]),
                        ("D_yt0", [1, 256]), ("D_y1s", [128, 256])]:
            dbg[nm] = nc.dram_tensor(nm, shp, F32, kind="ExternalOutput").ap()

    # DRAM scratch
    yD = nc.dram_tensor("yD", [128, NP, N], BF).ap()
    y1D = nc.dram_tensor("y1D", [2, 128, T, BC], BF).ap()

    with tile.TileContext(nc) as tc:
        with ExitStack() as ctx:
            _emit(ctx, tc, dram, OUT, yD, y1D, dbg)
    nc.compile()
    return nc


def _emit(ctx, tc, d, OUT, yD, y1D, dbg):
    nc = tc.nc
    sdma = nc.sync.dma_start      # big streams
    adma = nc.scalar.dma_start    # small/side transfers

    consts = ctx.enter_context(tc.tile_pool(name="consts", bufs=1))
    ypool = ctx.enter_context(tc.tile_pool(name="ypool", bufs=2 if DEBUG else 3))
    xtp = ctx.enter_context(tc.tile_pool(name="xtp", bufs=1))
    spool = ctx.enter_context(tc.tile_pool(name="spool", bufs=2))
    wpool = ctx.enter_context(tc.tile_pool(name="wpool", bufs=1))
    bfp = ctx.enter_context(tc.tile_pool(name="bfp", bufs=1))
    tpool = ctx.enter_context(tc.tile_pool(name="tpool", bufs=2))

    gps = ctx.enter_context(tc.tile_pool(name="gps", bufs=1, space="PSUM"))
    pAB = ctx.enter_context(tc.tile_pool(name="pAB", bufs=1, space="PSUM"))
    psm = ctx.enter_context(tc.tile_pool(name="psm", bufs=3, space="PSUM"))

    # ---- load constants into SBUF ----
    def cload(name, shape, dt, src_ap):
        t = consts.tile(shape, dt, tag=name)
        sdma(out=t[:], in_=src_ap)
        return t

    # [K, F] dram -> [128, K//128, F] sbuf (k-chunk layout)
    def kchunks(name, F, nk, dt=BF):
        t = consts.tile([128, nk, F], dt, tag=name)
        src = d[name]
        ap = bass.AP(tensor=src.tensor, offset=0,
                     ap=[[F, 128], [128 * F, nk], [1, F]])
        sdma(out=t[:], in_=ap)
        return t

    WeT_sb = kchunks("WeT", T, 4)
    encG_sb = kchunks("encG", 1024, 4)
    UdT_sb = kchunks("UdT", 256, 2)
    WdT_sb = kchunks("WdT", 256, 4)
    decG_sb = kchunks("decG", 1024, 2)
    wt2_sb = kchunks("wt2", 1, 2)
    WyT_sb = kchunks("WyT", 256, 4, dt=F32)

    UeT_sb = cload("UeT", [T, T], F32, d["UeT"][:])
    vzsel_sb = cload("vzsel", [128, 32, 64], BF, d["vzsel"][:])
    vdsel_sb = cload("vdsel", [128, 2, 32, 32], BF, d["vdsel"][:])
    Web_sb = cload("Web128", [128, 1], F32, d["Web128"][:])
    encGb_sb = cload("encGb", [1, 1024], BF, d["encGb"][:])
    WdTb_sb = cload("WdTb", [1, 256], BF, d["WdTb"][:])
    decG2_sb = cload("decG2", [2, 1024], BF, d["decG2"][:])
    WyTb_sb = cload("WyTb", [1, 256], F32, d["WyTb"][:])
    vyR_sb = cload("vyR", [128, 256], F32, d["vyR"][:])
    vyb_sb = cload("vyb128", [128, 1], F32, d["vyb128"][:])
    ident = cload("identD", [128, 128], BF, d["identD"][:])
    identF_sb = cload("identF", [128, 128], F32, d["identF"][:])

    ones1 = consts.tile([1, 256], BF, tag="ones1")
    nc.vector.memset(ones1[:], 1.0)
    ones32 = consts.tile([32, 1], BF, tag="ones32")
    nc.vector.memset(ones32[:], 1.0)
    ones1f = consts.tile([1, 128], F32, tag="ones1f")
    nc.vector.memset(ones1f[:], 1.0)
    ytil1 = consts.tile([2, 256], BF, tag="ytil1")
    sdma(out=ytil1[1:2, :], in_=ones1[0:1, :])   # DMA: engines can't write base partition 1
    # per-step beta selector: [p', group g, pair-in-group jj, m] with the
    # beta value at m = 2*jj + parity(p'); zeros elsewhere (memset once,
    # nonzero slots rewritten every decoder step).
    bbsel = consts.tile([128, 4, 32, 64], BF, tag="bbsel")
    nc.vector.memset(bbsel[:], 0.0)
    zT = consts.tile([128, 2, 256], BF, tag="zT")
    nc.vector.memset(zT[:], 0.0)
    c_st = consts.tile([128, 2, 256], F32, tag="c_st")
    nc.vector.memset(c_st[:], 0.0)
    s_st = consts.tile([128, 2, 256], F32, tag="s_st")
    nc.vector.memset(s_st[:], 0.0)
    Et = consts.tile([128, NP, 256], BF, tag="Et")

    X = d["X"]

    # ---- phase 0: y precompute -> yD ----
    # y[b, n, s] = sum_t Ue[s, t] X[b, t, n]; yD[p'=(s,par), i, n]
    with ExitStack() as pctx:
        xcp = pctx.enter_context(tc.tile_pool(name="xcp", bufs=1))
        for bq in range(64):
            # Xc[t, j', par, n] = X[bq*4 + 2j' + par, t, n]
            Xc = xcp.tile([T, 2, 2, N], F32, tag="Xc")
            src = bass.AP(tensor=X.tensor, offset=bq * 4 * T * N,
                          ap=[[N, T], [2 * T * N, 2], [T * N, 2], [1, N]])
            sdma(out=Xc[:], in_=src)
            yp = psm.tile([128, 512], F32, tag="sm")
            for par in (0, 1):
                nc.tensor.matmul(yp[par * 64:(par + 1) * 64, :],
                                 lhsT=UeT_sb[:], rhs=Xc[:, :, par, :],
                                 start=True, stop=True)
            ye = bfp.tile([128, 512], BF, tag="ye")
            nc.vector.tensor_copy(ye[:], yp[:])
            sdma(out=yD[:, bq * 2: bq * 2 + 2, :], in_=ye[:])

    # ---- encoder ----
    prev_hT = zT
    prev_cT = zT
    for t in range(T):
        # x_t
        xt = xtp.tile([128, 2, 256], F32, tag="xt")
        srcx = bass.AP(tensor=X.tensor, offset=t * N,
                       ap=[[T * N, 128], [128 * T * N, 2], [1, N]])
        adma(out=xt[:], in_=srcx)

        # x = We @ hc + We_b  ->  xs[p'=(s,par), i]
        xx = psm.tile([128, 128], F32, tag="sm")
        for par in (0, 1):
            for kc in range(4):
                hc = prev_hT if kc < 2 else prev_cT
                rhs = hc[:, kc % 2, par::2]
                nc.tensor.matmul(xx[par * 64:(par + 1) * 64, 0:128],
                                 lhsT=WeT_sb[:, kc, :], rhs=rhs,
                                 start=(kc == 0), stop=(kc == 3))
        xs = spool.tile([128, 128], F32, tag="xs")
        nc.vector.tensor_scalar_add(xs[:], xx[:, 0:128], Web_sb[:])

        # attention: stream y, add x, tanh, reduce with ve
        et = pAB.tile([128, 2, 256], F32, tag="pA")
        for c in range(8):
            yt8 = ypool.tile([128, 16, 256], BF, tag="ystream")
            sdma(out=yt8[:, 0:8, :], in_=yD[:, c * 16:c * 16 + 8, :])
            adma(out=yt8[:, 8:16, :], in_=yD[:, c * 16 + 8:(c + 1) * 16, :])
            for j in range(16):
                i = c * 16 + j
                nc.vector.tensor_scalar_add(yt8[:, j, :], yt8[:, j, :],
                                            xs[:, i:i + 1])
            nc.scalar.activation(yt8[:], yt8[:], AF.Tanh)
            for j in range(16):
                i = c * 16 + j
                g = i >> 5              # group of 32 pairs = 64 b rows
                jj = i & 31
                bh = i >> 6
                base = 64 * (g & 1)
                nc.tensor.matmul(et[base:base + 64, bh, :],
                                 lhsT=vzsel_sb[:, jj, :], rhs=yt8[:, j, :],
                                 start=(jj == 0), stop=(jj == 31))

        # softmax over n (no max-subtract; logits are small) and wx
        expe = wpool.tile([128, 2, 256], F32, tag="expe")
        nc.scalar.activation(expe[:], et[:], AF.Exp)
        ssum = wpool.tile([128, 2, 1], F32, tag="ssum")
        nc.vector.tensor_reduce(ssum[:], expe[:], mybir.AxisListType.X, OP.add)
        rs = wpool.tile([128, 2, 1], F32, tag="rs")
        nc.vector.reciprocal(rs[:], ssum[:])
        wxf = wpool.tile([128, 2, 256], F32, tag="wxf")
        nc.vector.tensor_mul(wxf[:], expe[:], xt[:])
        wxb = bfp.tile([128, 2, 256], BF, tag="wxb")
        for bh in (0, 1):
            nc.vector.tensor_scalar_mul(wxb[:, bh, :], wxf[:, bh, :],
                                        rs[:, bh, 0:1])

        # transposes: wxT[n-part, b], hT/cT built after pointwise
        wxT = tpool.tile([128, 2, 256], BF, tag="wxT")
        for bh in (0, 1):
            for nh in (0, 1):
                tp = psm.tile([128, 128], BF, tag="sm")
                nc.tensor.transpose(tp[:, 0:128],
                                    wxb[:, bh, nh * 128:(nh + 1) * 128], ident[:])
                nc.vector.tensor_copy(wxT[:, nh, bh * 128:(bh + 1) * 128],
                                      tp[:, 0:128])

        # gates + pointwise (both b-halves in one [128, 2, 1024] psum tile)
        h2b = bfp.tile([128, 2, 256], BF, tag="h2b")
        cbf = bfp.tile([128, 2, 256], BF, tag="cbf")
        g2 = gps.tile([128, 2, 1024], F32, tag="g")
        for bh in (0, 1):
            sl = slice(bh * 128, (bh + 1) * 128)
            lhss = [wxT[:, 0, sl], wxT[:, 1, sl],
                    prev_hT[:, 0, sl], prev_hT[:, 1, sl]]
            for kc in range(4):
                for nx in (0, 1):
                    nc.tensor.matmul(g2[:, bh, nx * 512:(nx + 1) * 512],
                                     lhsT=lhss[kc],
                                     rhs=encG_sb[:, kc, nx * 512:(nx + 1) * 512],
                                     start=(kc == 0), stop=False)
            for nx in (0, 1):
                nc.tensor.matmul(g2[:, bh, nx * 512:(nx + 1) * 512],
                                 lhsT=ones1[0:1, 0:128],
                                 rhs=encGb_sb[0:1, nx * 512:(nx + 1) * 512],
                                 start=False, stop=True)
        _lstm_pointwise(nc, wpool, g2, c_st[:], h2b[:], cbf[:])

        hT = tpool.tile([128, 2, 256], BF, tag="hT")
        cT = tpool.tile([128, 2, 256], BF, tag="cT")
        for (src, dst) in ((h2b, hT), (cbf, cT)):
            for bh in (0, 1):
                for mh in (0, 1):
                    tp = psm.tile([128, 128], BF, tag="sm")
                    nc.tensor.transpose(tp[:, 0:128],
                                        src[:, bh, mh * 128:(mh + 1) * 128],
                                        ident[:])
                    nc.vector.tensor_copy(dst[:, mh, bh * 128:(bh + 1) * 128],
                                          tp[:, 0:128])

        # Et scatter: Et[t + 64*par, bh*64 + k, :] = h2[2k+par (in bh), :]
        # (gpsimd SWDGE queue: keeps the HWDGE queues free for the y streams)
        for bh in (0, 1):
            for par in (0, 1):
                nc.gpsimd.dma_start(
                    out=Et[t + 64 * par: t + 64 * par + 1,
                           bh * 64:(bh + 1) * 64, :],
                    in_=h2b[par::2, bh, :])

        # y1 increment: y1D[mh, :, t, :] = UdT[:, :, mh] @ hT
        for mh in (0, 1):
            yp1 = psm.tile([128, 512], F32, tag="sm")
            for kc in (0, 1):
                nc.tensor.matmul(yp1[:, 0:256],
                                 lhsT=UdT_sb[:, kc, mh * 128:(mh + 1) * 128],
                                 rhs=hT[:, kc, :], start=(kc == 0), stop=(kc == 1))
            y1e = bfp.tile([128, 256], BF, tag="y1e")
            nc.vector.tensor_copy(y1e[:], yp1[:, 0:256])
            sdma(out=y1D[mh, :, t, :], in_=y1e[:])

        if DEBUG and t == 0:
            adma(out=dbg["D_xs0"][:], in_=xs[:])
            de = wpool.tile([128, 256], F32, tag="de")
            nc.vector.tensor_copy(de[:], et[:, 0, :])
            adma(out=dbg["D_e0"][:], in_=de[:])
            adma(out=dbg["D_wx0"][:], in_=wxf[:])
            dh = wpool.tile([128, 2, 256], F32, tag="dh")
            nc.vector.tensor_copy(dh[:], h2b[:])
            adma(out=dbg["D_h0"][:], in_=dh[:])
            dy1 = wpool.tile([128, 256], F32, tag="dy1")
            nc.vector.tensor_copy(dy1[:], y1e[:])
            adma(out=dbg["D_y1s"][:], in_=dy1[:])
        if DEBUG and t == T - 1:
            dh = wpool.tile([128, 2, 256], F32, tag="dh")
            nc.vector.tensor_copy(dh[:], h2b[:])
            adma(out=dbg["D_hf"][:], in_=dh[:])
            adma(out=dbg["D_cf"][:], in_=c_st[:])

        prev_hT, prev_cT = hT, cT

    # ---- decoder ----
    prev_dT = zT
    prev_sT = zT
    last_ctT = None
    for t in range(T):
        # x1 = Wd @ [d; s] + Wd_b   -> x1T[m-part(mh), b]
        x1p = psm.tile([128, 512], F32, tag="sm")
        for mh in (0, 1):
            o = x1p[:, mh * 256:(mh + 1) * 256]
            for kc in range(4):
                ds_ = prev_dT if kc < 2 else prev_sT
                nc.tensor.matmul(o, lhsT=WdT_sb[:, kc, mh * 128:(mh + 1) * 128],
                                 rhs=ds_[:, kc % 2, :], start=(kc == 0), stop=False)
            nc.tensor.matmul(o, lhsT=WdTb_sb[0:1, mh * 128:(mh + 1) * 128],
                             rhs=ones1[0:1, 0:256], start=False, stop=True)
        x1b = bfp.tile([128, 2, 256], BF, tag="x1b")
        nc.vector.tensor_copy(x1b[:], x1p[:])

        # gate matmuls that only need prev_dT: emitted early so PE can run
        # them while the attention softmax / bbsel scatter is in flight
        g2 = gps.tile([128, 2, 1024], F32, tag="g")
        for bh in (0, 1):
            sl = slice(bh * 128, (bh + 1) * 128)
            for kc in (0, 1):
                for nx in (0, 1):
                    nc.tensor.matmul(g2[:, bh, nx * 512:(nx + 1) * 512],
                                     lhsT=prev_dT[:, kc, sl],
                                     rhs=decG_sb[:, kc, nx * 512:(nx + 1) * 512],
                                     start=(kc == 0), stop=False)

        # attention: stream y1, add x1 (broadcast over t), tanh, vd-reduce
        l_ps = pAB.tile([32, 512], F32, tag="pA")
        for ci, (tc16, mh) in enumerate([(a, b) for a in range(4) for b in (0, 1)]):
            z8 = ypool.tile([128, 16, 256], BF, tag="ystream")
            sdma(out=z8[:, 0:8, :], in_=y1D[mh, :, tc16 * 16:tc16 * 16 + 8, :])
            adma(out=z8[:, 8:16, :], in_=y1D[mh, :, tc16 * 16 + 8:(tc16 + 1) * 16, :])
            nc.vector.tensor_tensor(z8[:], z8[:],
                                    _bcast(x1b[:, mh, :], 16, 1), OP.add)
            nc.scalar.activation(z8[:], z8[:], AF.Tanh)
            for q in range(8):
                r = tc16 * 8 + q
                nc.tensor.matmul(l_ps[:],
                                 lhsT=vdsel_sb[:, mh, r, :],
                                 rhs=z8[:, 2 * q:2 * q + 2, :],
                                 start=(ci == 0 and q == 0),
                                 stop=(ci == 7 and q == 7))

        # softmax over t (no max-subtract).  l_ps rows r = flat(t*256+b)/512,
        # i.e. row r holds t in {2r, 2r+1} (col block tl) for all b.
        expl = bfp.tile([32, 512], BF, tag="expl")
        nc.scalar.activation(expl[:], l_ps[:], AF.Exp)
        # per-b sums: ones-matmul over the 32 rows, then fold the tl pairs
        sum_ps = pAB.tile([32, 512], F32, tag="pA")
        nc.tensor.matmul(sum_ps[0:1, :], lhsT=ones32[:], rhs=expl[:],
                         start=True, stop=True)
        ssum1 = wpool.tile([1, 256], F32, tag="ssum1")
        sview = bass.AP(tensor=sum_ps[:].tensor, offset=sum_ps[:].offset,
                        ap=[[512, 1], [1, 256], [256, 2]])
        nc.vector.tensor_reduce(ssum1[:], sview, mybir.AxisListType.X, OP.add)
        rs2 = wpool.tile([128, 2, 1], F32, tag="rs2")
        for bh in (0, 1):
            adma(out=rs2[:, bh, :], in_=ssum1[0:1, bh * 128:(bh + 1) * 128])
        nc.vector.reciprocal(rs2[:], rs2[:])
        # scatter UNnormalized exp(l) into the block-diag selector (c_t is
        # normalized at eviction with the f32 1/sum — avoids a systematic
        # bf16 scale error on c_t):
        # bbsel[p'=2c+tl+64par, g, jj, 2jj+par] = exp(l)(b=2(32g+jj)+par, t=2c+tl)
        for par in (0, 1):
            for tl in (0, 1):
                p0 = tl + 64 * par
                for g in range(4):
                    dst = bass.AP(tensor=bbsel[:].tensor,
                                  offset=bbsel[:].offset + p0 * 8192
                                  + g * 2048 + par,
                                  ap=[[2 * 8192, 32], [66, 32]])
                    c0 = tl * 256 + par + 64 * g
                    (sdma if g % 2 == 0 else adma)(
                        out=dst, in_=expl[:, c0: c0 + 63: 2])

        # c_t: selector matmuls, 32-pair accumulation groups
        ctp = pAB.tile([128, 2, 256], F32, tag="pA")
        for i in range(NP):
            g = i >> 5
            jj = i & 31
            bh = i >> 6
            base = 64 * (g & 1)
            nc.tensor.matmul(ctp[base:base + 64, bh, :],
                             lhsT=bbsel[:, g, jj, :], rhs=Et[:, i, :],
                             start=(jj == 0), stop=(jj == 31))
        ctb = bfp.tile([128, 2, 256], BF, tag="ctb")
        nc.vector.tensor_scalar_mul(ctb[:, 0, :], ctp[:, 0, :], rs2[:, 0, 0:1])
        nc.vector.tensor_scalar_mul(ctb[:, 1, :], ctp[:, 1, :], rs2[:, 1, 0:1])

        ctT = tpool.tile([128, 2, 256], BF, tag="ctT")
        for bh in (0, 1):
            for mh in (0, 1):
                tp = psm.tile([128, 128], BF, tag="sm")
                nc.tensor.transpose(tp[:, 0:128],
                                    ctb[:, bh, mh * 128:(mh + 1) * 128], ident[:])
                nc.vector.tensor_copy(ctT[:, mh, bh * 128:(bh + 1) * 128],
                                      tp[:, 0:128])

        # y_til = wt @ c_t (wt_b folded into decG2 bias row)
        ytp = pAB.tile([1, 512], F32, tag="pA")
        for mh in (0, 1):
            nc.tensor.matmul(ytp[:, 0:256], lhsT=wt2_sb[:, mh, :],
                             rhs=ctT[:, mh, :], start=(mh == 0), stop=(mh == 1))
        nc.vector.tensor_copy(ytil1[0:1, :], ytp[:, 0:256])

        # decoder LSTM
        d2b = bfp.tile([128, 2, 256], BF, tag="d2b")
        sbf = bfp.tile([128, 2, 256], BF, tag="sbf")
        final = (t == T - 1)
        if final:
            d2f = wpool.tile([128, 2, 256], F32, tag="d2f")
            ctf = wpool.tile([128, 2, 256], F32, tag="ctf")
            nc.vector.tensor_scalar_mul(ctf[:, 0, :], ctp[:, 0, :], rs2[:, 0, 0:1])
            nc.vector.tensor_scalar_mul(ctf[:, 1, :], ctp[:, 1, :], rs2[:, 1, 0:1])
        for bh in (0, 1):
            sl = slice(bh * 128, (bh + 1) * 128)
            for nx in (0, 1):
                nc.tensor.matmul(g2[:, bh, nx * 512:(nx + 1) * 512],
                                 lhsT=ytil1[0:2, sl],
                                 rhs=decG2_sb[0:2, nx * 512:(nx + 1) * 512],
                                 start=False, stop=True)
        _lstm_pointwise(nc, wpool, g2, s_st[:], d2b[:], sbf[:],
                        h_out_f32=d2f[:] if final else None)

        dT = tpool.tile([128, 2, 256], BF, tag="dT")
        sT = tpool.tile([128, 2, 256], BF, tag="sT")
        for (src, dst) in ((d2b, dT), (sbf, sT)):
            for bh in (0, 1):
                for mh in (0, 1):
                    tp = psm.tile([128, 128], BF, tag="sm")
                    nc.tensor.transpose(tp[:, 0:128],
                                        src[:, bh, mh * 128:(mh + 1) * 128],
                                        ident[:])
                    nc.vector.tensor_copy(dst[:, mh, bh * 128:(bh + 1) * 128],
                                          tp[:, 0:128])

        if DEBUG and t == 0:
            dl = wpool.tile([32, 512], F32, tag="dl")
            nc.vector.tensor_copy(dl[:], expl[:])
            adma(out=dbg["D_expl0"][:], in_=dl[:])
            dct = wpool.tile([128, 2, 256], F32, tag="dct")
            nc.vector.tensor_copy(dct[:], ctb[:])
            adma(out=dbg["D_ct0"][:], in_=dct[:])
            dyt = wpool.tile([1, 256], F32, tag="dyt")
            nc.vector.tensor_copy(dyt[:], ytil1[0:1, :])
            adma(out=dbg["D_yt0"][:], in_=dyt[:])

        prev_dT, prev_sT = dT, sT
        last_ctT = ctT
        if final:
            # f32 transposes of the final d and c_t for the f32 head
            dTf = wpool.tile([128, 2, 256], F32, tag="dTf")
            cTf = wpool.tile([128, 2, 256], F32, tag="cTf")
            for (src, dst) in ((d2f, dTf), (ctf, cTf)):
                for bh in (0, 1):
                    for mh in (0, 1):
                        tp = psm.tile([128, 512], F32, tag="sm")
                        nc.tensor.transpose(tp[:, 0:128],
                                            src[:, bh, mh * 128:(mh + 1) * 128],
                                            identF_sb[:])
                        nc.vector.tensor_copy(dst[:, mh, bh * 128:(bh + 1) * 128],
                                              tp[:, 0:128])
            head_dT, head_cT = dTf, cTf

    # ---- head (f32): out = (dc @ Wy.T + Wy_b) @ vy.T + vy_b ----
    for bh in (0, 1):
        sl = slice(bh * 128, (bh + 1) * 128)
        o1 = psm.tile([128, 512], F32, tag="sm")
        for kc in range(4):
            dc = head_dT if kc < 2 else head_cT
            nc.tensor.matmul(o1[:, 0:256], lhsT=dc[:, kc % 2, sl],
                             rhs=WyT_sb[:, kc, :], start=(kc == 0), stop=False)
        nc.tensor.matmul(o1[:, 0:256], lhsT=ones1f[0:1, 0:128],
                         rhs=WyTb_sb[:], start=False, stop=True)
        tmp = wpool.tile([128, 256], F32, tag="tmp")
        nc.vector.tensor_mul(tmp[:], o1[:, 0:256], vyR_sb[:])
        red = wpool.tile([128, 1], F32, tag="red")
        nc.vector.tensor_reduce(red[:], tmp[:], mybir.AxisListType.X, OP.add)
        ob = wpool.tile([128, 1], F32, tag="ob")
        nc.vector.tensor_scalar_add(ob[:], red[:], vyb_sb[:])
        adma(out=OUT[bh * 128:(bh + 1) * 128, :], in_=ob[:])


def _lstm_pointwise(nc, wpool, g2, c_ap, h_out, c_out_bf, h_out_f32=None):
    """PyTorch LSTMCell pointwise from gate preacts g2 [128, 2(bh), 1024]
    (psum), both b-halves at once via strided [128, 2, .] APs.

    sigmoid(x) = 0.5*tanh(x/2)+0.5.  c_ap [128, 2, 256] updated in place
    (f32); h_out and c_out_bf are bf16 [128, 2, 256] destinations.
    """
    AFt = AF.Tanh
    tif = wpool.tile([128, 2, 512], F32, tag="tif")
    nc.scalar.activation(tif[:], g2[:, :, 0:512], AFt, scale=0.5)
    tg = wpool.tile([128, 2, 256], F32, tag="tg")
    nc.scalar.activation(tg[:], g2[:, :, 512:768], AFt)
    to = wpool.tile([128, 2, 256], F32, tag="to")
    nc.scalar.activation(to[:], g2[:, :, 768:1024], AFt, scale=0.5)
    # sigmoid in place: tif <- (tif+1)*0.5
    nc.vector.tensor_scalar(tif[:], tif[:], 1.0, 0.5, op0=OP.add, op1=OP.mult)
    e1 = wpool.tile([128, 2, 256], F32, tag="e1")
    nc.vector.tensor_mul(e1[:], tif[:, :, 256:512], c_ap)
    nc.vector.tensor_mul(tg[:], tif[:, :, 0:256], tg[:])     # in place
    nc.vector.tensor_add(c_ap, e1[:], tg[:])
    tc2 = wpool.tile([128, 2, 256], F32, tag="tc2")
    nc.scalar.activation(tc2[:], c_ap, AFt)
    nc.vector.tensor_scalar(to[:], to[:], 1.0, 0.5, op0=OP.add, op1=OP.mult)
    nc.vector.tensor_mul(h_out, to[:], tc2[:])
    if h_out_f32 is not None:
        nc.vector.tensor_mul(h_out_f32, to[:], tc2[:])
    nc.vector.tensor_copy(c_out_bf, c_ap)


def _prep(inp):
    """Host-side weight prep (tiny tensors only). Returns per-core common map."""
    f32 = np.float32

    def b(x):
        return np.ascontiguousarray(np.asarray(x, f32).astype(bf16))

    ve = np.asarray(inp["ve_w"], f32)[0]          # [T]
    # vzsel[p', jj, m] = ve[p'%64] iff m == 2*jj + (p'>=64): block-diag
    # selector so 32 pair-matmuls accumulate into distinct row pairs.
    vzsel = np.zeros((128, 32, 64), f32)
    for jj in range(32):
        vzsel[0:64, jj, 2 * jj] = ve
        vzsel[64:128, jj, 2 * jj + 1] = ve
    vd = np.asarray(inp["vd_w"], f32)[0]          # [M]
    vdsel = np.zeros((128, 2, 32, 32), f32)
    for r in range(32):
        vdsel[:, 0, r, r] = vd[0:128]
        vdsel[:, 1, r, r] = vd[128:256]

    com = {
        "UeT": np.ascontiguousarray(np.asarray(inp["Ue_w"], f32).T),
        "vzsel": b(vzsel),
        "vdsel": b(vdsel),
        "WeT": b(np.asarray(inp["We_w"], f32).T),
        "Web128": np.ascontiguousarray(
            np.tile(np.asarray(inp["We_b"], f32), 2)[:, None]),
        "encG": b(np.concatenate([np.asarray(inp["enc_Wih"], f32).T,
                                  np.asarray(inp["enc_Whh"], f32).T], axis=0)),
        "encGb": b((np.asarray(inp["enc_bih"], f32)
                    + np.asarray(inp["enc_bhh"], f32))[None, :]),
        "UdT": b(np.asarray(inp["Ud_w"], f32).T),
        "WdT": b(np.asarray(inp["Wd_w"], f32).T),
        "WdTb": b(np.asarray(inp["Wd_b"], f32)[None, :]),
        "decG": b(np.asarray(inp["dec_Whh"], f32).T),
        "decG2": b(np.stack([
            np.asarray(inp["dec_Wih"], f32)[:, 0],
            np.asarray(inp["dec_bih"], f32) + np.asarray(inp["dec_bhh"], f32)
            + float(np.asarray(inp["wt_b"], f32)[0])
            * np.asarray(inp["dec_Wih"], f32)[:, 0]], axis=0)),
        "wt2": b(np.asarray(inp["wt_w"], f32)[0][:, None]),
        "WyT": np.ascontiguousarray(np.asarray(inp["Wy_w"], f32).T),
        "WyTb": np.ascontiguousarray(np.asarray(inp["Wy_b"], f32)[None, :]),
        "vyR": np.ascontiguousarray(
            np.tile(np.asarray(inp["vy_w"], f32), (128, 1))),
        "vyb128": np.full((128, 1), float(np.asarray(inp["vy_b"], f32)[0]), f32),
        "identD": np.eye(128, dtype=f32).astype(bf16),
        "identF": np.eye(128, dtype=f32),
    }
    return com


def kernel(**inputs):
    global _CACHED_NC
    if _CACHED_NC is None:
        _CACHED_NC = build_nc()
    nc = _CACHED_NC

    com = _prep(inputs)
    Xfull = np.ascontiguousarray(np.asarray(inputs["X_history"], np.float32))
    in_maps = []
    for c in range(NCORES):
        m = dict(com)
        m["X"] = np.ascontiguousarray(Xfull[c * BC:(c + 1) * BC])
        in_maps.append(m)

    trace = bool(int(os.environ.get("DARNN_TRACE", "0")))
    r = run_bass_kernel_spmd(nc, in_maps, list(range(NCORES)), trace=trace)
    res = r.results
    if trace:
        kernel._last_exec_ns = r.exec_time_ns
        kernel._last_profile = r.profile_json
        kernel._trace = r.instructions_and_trace
    out = np.concatenate([res[c]["OUT"] for c in range(NCORES)], axis=0)
    if DEBUG:
        kernel._dbg = res
    return out.astype(np.float32)


# revision 54
# speedup vs baseline: 1.0034x; 1.0034x over previous
"""DA-RNN (dual-stage attention RNN) Trainium2 Bass kernel.

Problem: nn_Darnn_14774687498323.  B=2048, T=64, N=256, M=P=256.
Sharding: pure data parallel, batch split 8 ways (BC=256 per core), weights
replicated.  Full inputs in, full outputs out.

Per-core layout conventions
---------------------------
b       : per-core batch index, 0..255.  b-half bh = b // 128.
pair i  : i = b >> 1 (128 pairs); parity = b & 1.
p'      : "pair partition" = s + 64*parity packs the 64 attention positions
          of two consecutive batch rows into 128 partitions.

Encoder attention:  e[b,n] = sum_s ve[s] * tanh(x[b,s] + y[b,n,s]) with
y = einsum('btn,st->bns').  y is precomputed to DRAM in the [p', i, n]
layout and streamed each step; the x-add runs as per-partition-scalar DVE
adds, tanh as bulk in-place ACT, and the ve-sum as one tiny PE matmul per
pair with a block-diagonal 2-column ve (out = e rows [2i, 2i+1]).

Decoder attention: l[b,t] = sum_m vd[m] * tanh(x1[b,m] + y1[b,t,m]).
y1 is built incrementally during the encoder in the [m-half][mp, t, b]
layout (b innermost) and streamed each decoder step; the x1-add runs as
bf16 tensor_tensor with an outer-dim-0 broadcast AP, the vd-sum as clean
K=128 matmuls into an l-psum of shape [32, 512] = [flat(t,b)/512, .].
Softmax-over-t sums come from a ones-vector matmul + strided DVE reduce;
unnormalized exp(l) is scattered into the block-diag buffer for the c_t
pair-matmuls and normalization is folded into the c_t psum eviction.

sigmoid(x) = 0.5*tanh(x/2) + 0.5 everywhere so the ACT table set never
leaves exp_and_others (tanh + exp), avoiding ~2.7us table reloads.
"""

import os
import numpy as np
import ml_dtypes
from contextlib import ExitStack

import concourse.bass as bass
import concourse.tile as tile
from concourse import bacc, mybir
from concourse.bass_utils import run_bass_kernel_spmd

F32 = mybir.dt.float32
BF = mybir.dt.bfloat16
bf16 = ml_dtypes.bfloat16
AF = mybir.ActivationFunctionType
OP = mybir.AluOpType

T, N, M, B = 64, 256, 256, 2048
NCORES = 8
BC = B // NCORES          # 256 batch rows per core
NP = BC // 2              # 128 pairs

DEBUG = bool(int(os.environ.get("DARNN_DEBUG", "0")))

_CACHED_NC = None


def _bcast(ap, n, axis):
    """Insert a step-0 dim of size n at free-dim position `axis` (1-based in ap list)."""
    new = list(ap.ap)
    new.insert(axis, [0, n])
    return bass.AP(tensor=ap.tensor, offset=ap.offset, ap=new)


def build_nc():
    nc = bacc.Bacc("TRN2", target_bir_lowering=False, debug=False,
                   num_devices=NCORES)

    dram = {}

    def din(name, shape, dt):
        dram[name] = nc.dram_tensor(name, shape, dt, kind="ExternalInput").ap()
        return dram[name]

    X = din("X", [BC, T, N], F32)
    UeT = din("UeT", [T, T], F32)
    vzsel = din("vzsel", [128, 32, 64], BF)
    WeT = din("WeT", [512, T], BF)
    Web128 = din("Web128", [128, 1], F32)
    encG = din("encG", [512, 1024], BF)
    encGb = din("encGb", [1, 1024], BF)
    UdT = din("UdT", [256, 256], BF)
    vdsel = din("vdsel", [128, 2, 32, 32], BF)
    WdT = din("WdT", [512, 256], BF)
    WdTb = din("WdTb", [1, 256], BF)
    decG = din("decG", [256, 1024], BF)
    decG2 = din("decG2", [2, 1024], BF)
    wt2 = din("wt2", [256, 1], BF)
    WyT = din("WyT", [512, 256], F32)
    WyTb = din("WyTb", [1, 256], F32)
    vyR = din("vyR", [128, 256], F32)
    vyb128 = din("vyb128", [128, 1], F32)
    identD = din("identD", [128, 128], BF)
    identF = din("identF", [128, 128], F32)

    OUT = nc.dram_tensor("OUT", [BC, 1], F32, kind="ExternalOutput").ap()
    dbg = {}
    if DEBUG:
        for nm, shp in [("D_xs0", [128, 128]), ("D_e0", [128, 256]),
                        ("D_wx0", [128, 2, 256]), ("D_h0", [128, 2, 256]),
                        ("D_hf", [128, 2, 256]), ("D_cf", [128, 2, 256]),
                        ("D_expl0", [32, 512]), ("D_ct0", [128, 2, 256]),
                        ("D_yt0", [1, 256]), ("D_y1s", [128, 256])]:
            dbg[nm] = nc.dram_tensor(nm, shp, F32, kind="ExternalOutput").ap()

    # DRAM scratch
    yD = nc.dram_tensor("yD", [128, NP, N], BF).ap()
    y1D = nc.dram_tensor("y1D", [2, 128, T, BC], BF).ap()

    with tile.TileContext(nc) as tc:
        with ExitStack() as ctx:
            _emit(ctx, tc, dram, OUT, yD, y1D, dbg)
    nc.compile()
    return nc


def _emit(ctx, tc, d, OUT, yD, y1D, dbg):
    nc = tc.nc
    sdma = nc.sync.dma_start      # big streams
    adma = nc.scalar.dma_start    # small/side transfers

    consts = ctx.enter_context(tc.tile_pool(name="consts", bufs=1))
    ypool = ctx.enter_context(tc.tile_pool(name="ypool", bufs=2 if DEBUG else 3))
    xtp = ctx.enter_context(tc.tile_pool(name="xtp", bufs=1))
    spool = ctx.enter_context(tc.tile_pool(name="spool", bufs=2))
    wpool = ctx.enter_context(tc.tile_pool(name="wpool", bufs=1))
    bfp = ctx.enter_context(tc.tile_pool(name="bfp", bufs=1))
    tpool = ctx.enter_context(tc.tile_pool(name="tpool", bufs=2))

    gps = ctx.enter_context(tc.tile_pool(name="gps", bufs=1, space="PSUM"))
    pAB = ctx.enter_context(tc.tile_pool(name="pAB", bufs=1, space="PSUM"))
    psm = ctx.enter_context(tc.tile_pool(name="psm", bufs=3, space="PSUM"))

    # ---- load constants into SBUF ----
    def cload(name, shape, dt, src_ap):
        t = consts.tile(shape, dt, tag=name)
        sdma(out=t[:], in_=src_ap)
        return t

    # [K, F] dram -> [128, K//128, F] sbuf (k-chunk layout)
    def kchunks(name, F, nk, dt=BF):
        t = consts.tile([128, nk, F], dt, tag=name)
        src = d[name]
        ap = bass.AP(tensor=src.tensor, offset=0,
                     ap=[[F, 128], [128 * F, nk], [1, F]])
        sdma(out=t[:], in_=ap)
        return t

    WeT_sb = kchunks("WeT", T, 4)
    encG_sb = kchunks("encG", 1024, 4)
    UdT_sb = kchunks("UdT", 256, 2)
    WdT_sb = kchunks("WdT", 256, 4)
    decG_sb = kchunks("decG", 1024, 2)
    wt2_sb = kchunks("wt2", 1, 2)
    WyT_sb = kchunks("WyT", 256, 4, dt=F32)

    UeT_sb = cload("UeT", [T, T], F32, d["UeT"][:])
    vzsel_sb = cload("vzsel", [128, 32, 64], BF, d["vzsel"][:])
    vdsel_sb = cload("vdsel", [128, 2, 32, 32], BF, d["vdsel"][:])
    Web_sb = cload("Web128", [128, 1], F32, d["Web128"][:])
    encGb_sb = cload("encGb", [1, 1024], BF, d["encGb"][:])
    WdTb_sb = cload("WdTb", [1, 256], BF, d["WdTb"][:])
    decG2_sb = cload("decG2", [2, 1024], BF, d["decG2"][:])
    WyTb_sb = cload("WyTb", [1, 256], F32, d["WyTb"][:])
    vyR_sb = cload("vyR", [128, 256], F32, d["vyR"][:])
    vyb_sb = cload("vyb128", [128, 1], F32, d["vyb128"][:])
    ident = cload("identD", [128, 128], BF, d["identD"][:])
    identF_sb = cload("identF", [128, 128], F32, d["identF"][:])

    ones1 = consts.tile([1, 256], BF, tag="ones1")
    nc.vector.memset(ones1[:], 1.0)
    ones32 = consts.tile([32, 1], BF, tag="ones32")
    nc.vector.memset(ones32[:], 1.0)
    ones1f = consts.tile([1, 128], F32, tag="ones1f")
    nc.vector.memset(ones1f[:], 1.0)
    ytil1 = consts.tile([2, 256], BF, tag="ytil1")
    sdma(out=ytil1[1:2, :], in_=ones1[0:1, :])   # DMA: engines can't write base partition 1
    # per-step beta selector: [p', group g, pair-in-group jj, m] with the
    # beta value at m = 2*jj + parity(p'); zeros elsewhere (memset once,
    # nonzero slots rewritten every decoder step).
    bbsel = consts.tile([128, 4, 32, 64], BF, tag="bbsel")
    nc.vector.memset(bbsel[:], 0.0)
    zT = consts.tile([128, 2, 256], BF, tag="zT")
    nc.vector.memset(zT[:], 0.0)
    c_st = consts.tile([128, 2, 256], F32, tag="c_st")
    nc.vector.memset(c_st[:], 0.0)
    s_st = consts.tile([128, 2, 256], F32, tag="s_st")
    nc.vector.memset(s_st[:], 0.0)
    Et = consts.tile([128, NP, 256], BF, tag="Et")

    X = d["X"]

    # ---- phase 0: y precompute -> yD ----
    # y[b, n, s] = sum_t Ue[s, t] X[b, t, n]; yD[p'=(s,par), i, n]
    with ExitStack() as pctx:
        xcp = pctx.enter_context(tc.tile_pool(name="xcp", bufs=1))
        for bq in range(64):
            # Xc[t, j', par, n] = X[bq*4 + 2j' + par, t, n]
            Xc = xcp.tile([T, 2, 2, N], F32, tag="Xc")
            src = bass.AP(tensor=X.tensor, offset=bq * 4 * T * N,
                          ap=[[N, T], [2 * T * N, 2], [T * N, 2], [1, N]])
            sdma(out=Xc[:], in_=src)
            yp = psm.tile([128, 512], F32, tag="sm")
            for par in (0, 1):
                nc.tensor.matmul(yp[par * 64:(par + 1) * 64, :],
                                 lhsT=UeT_sb[:], rhs=Xc[:, :, par, :],
                                 start=True, stop=True)
            ye = bfp.tile([128, 512], BF, tag="ye")
            nc.vector.tensor_copy(ye[:], yp[:])
            sdma(out=yD[:, bq * 2: bq * 2 + 2, :], in_=ye[:])

    # ---- encoder ----
    prev_hT = zT
    prev_cT = zT
    for t in range(T):
        # x_t
        xt = xtp.tile([128, 2, 256], F32, tag="xt")
        srcx = bass.AP(tensor=X.tensor, offset=t * N,
                       ap=[[T * N, 128], [128 * T * N, 2], [1, N]])
        nc.gpsimd.dma_start(out=xt[:], in_=srcx)

        # x = We @ hc + We_b  ->  xs[p'=(s,par), i]
        xx = psm.tile([128, 128], F32, tag="sm")
        for par in (0, 1):
            for kc in range(4):
                hc = prev_hT if kc < 2 else prev_cT
                rhs = hc[:, kc % 2, par::2]
                nc.tensor.matmul(xx[par * 64:(par + 1) * 64, 0:128],
                                 lhsT=WeT_sb[:, kc, :], rhs=rhs,
                                 start=(kc == 0), stop=(kc == 3))
        xs = spool.tile([128, 128], F32, tag="xs")
        nc.vector.tensor_scalar_add(xs[:], xx[:, 0:128], Web_sb[:])

        # attention: stream y, add x, tanh, reduce with ve
        et = pAB.tile([128, 2, 256], F32, tag="pA")
        for c in range(8):
            yt8 = ypool.tile([128, 16, 256], BF, tag="ystream")
            sdma(out=yt8[:, 0:8, :], in_=yD[:, c * 16:c * 16 + 8, :])
            adma(out=yt8[:, 8:16, :], in_=yD[:, c * 16 + 8:(c + 1) * 16, :])
            for j in range(16):
                i = c * 16 + j
                nc.vector.tensor_scalar_add(yt8[:, j, :], yt8[:, j, :],
                                            xs[:, i:i + 1])
            nc.scalar.activation(yt8[:], yt8[:], AF.Tanh)
            for j in range(16):
                i = c * 16 + j
                g = i >> 5              # group of 32 pairs = 64 b rows
                jj = i & 31
                bh = i >> 6
                base = 64 * (g & 1)
                nc.tensor.matmul(et[base:base + 64, bh, :],
                                 lhsT=vzsel_sb[:, jj, :], rhs=yt8[:, j, :],
                                 start=(jj == 0), stop=(jj == 31))

        # softmax over n (no max-subtract; logits are small) and wx
        expe = wpool.tile([128, 2, 256], F32, tag="expe")
        nc.scalar.activation(expe[:], et[:], AF.Exp)
        ssum = wpool.tile([128, 2, 1], F32, tag="ssum")
        nc.vector.tensor_reduce(ssum[:], expe[:], mybir.AxisListType.X, OP.add)
        rs = wpool.tile([128, 2, 1], F32, tag="rs")
        nc.vector.reciprocal(rs[:], ssum[:])
        wxb = bfp.tile([128, 2, 256], BF, tag="wxb")
        for bh in (0, 1):
            nc.vector.scalar_tensor_tensor(wxb[:, bh, :], expe[:, bh, :],
                                           rs[:, bh, 0:1], xt[:, bh, :],
                                           op0=OP.mult, op1=OP.mult)

        # transposes: wxT[n-part, b], hT/cT built after pointwise
        wxT = tpool.tile([128, 2, 256], BF, tag="wxT")
        for bh in (0, 1):
            for nh in (0, 1):
                tp = psm.tile([128, 128], BF, tag="sm")
                nc.tensor.transpose(tp[:, 0:128],
                                    wxb[:, bh, nh * 128:(nh + 1) * 128], ident[:])
                nc.vector.tensor_copy(wxT[:, nh, bh * 128:(bh + 1) * 128],
                                      tp[:, 0:128])

        # gates + pointwise (both b-halves in one [128, 2, 1024] psum tile)
        h2b = bfp.tile([128, 2, 256], BF, tag="h2b")
        cbf = bfp.tile([128, 2, 256], BF, tag="cbf")
        g2 = gps.tile([128, 2, 1024], F32, tag="g")
        for bh in (0, 1):
            sl = slice(bh * 128, (bh + 1) * 128)
            lhss = [wxT[:, 0, sl], wxT[:, 1, sl],
                    prev_hT[:, 0, sl], prev_hT[:, 1, sl]]
            for kc in range(4):
                for nx in (0, 1):
                    nc.tensor.matmul(g2[:, bh, nx * 512:(nx + 1) * 512],
                                     lhsT=lhss[kc],
                                     rhs=encG_sb[:, kc, nx * 512:(nx + 1) * 512],
                                     start=(kc == 0), stop=False)
            for nx in (0, 1):
                nc.tensor.matmul(g2[:, bh, nx * 512:(nx + 1) * 512],
                                 lhsT=ones1[0:1, 0:128],
                                 rhs=encGb_sb[0:1, nx * 512:(nx + 1) * 512],
                                 start=False, stop=True)
        _lstm_pointwise(nc, wpool, g2, c_st[:], h2b[:], cbf[:])

        hT = tpool.tile([128, 2, 256], BF, tag="hT")
        cT = tpool.tile([128, 2, 256], BF, tag="cT")
        for (src, dst) in ((h2b, hT), (cbf, cT)):
            for bh in (0, 1):
                for mh in (0, 1):
                    tp = psm.tile([128, 128], BF, tag="sm")
                    nc.tensor.transpose(tp[:, 0:128],
                                        src[:, bh, mh * 128:(mh + 1) * 128],
                                        ident[:])
                    nc.vector.tensor_copy(dst[:, mh, bh * 128:(bh + 1) * 128],
                                          tp[:, 0:128])

        # Et scatter: Et[t + 64*par, bh*64 + k, :] = h2[2k+par (in bh), :]
        # (gpsimd SWDGE queue: keeps the HWDGE queues free for the y streams)
        for bh in (0, 1):
            for par in (0, 1):
                nc.gpsimd.dma_start(
                    out=Et[t + 64 * par: t + 64 * par + 1,
                           bh * 64:(bh + 1) * 64, :],
                    in_=h2b[par::2, bh, :])

        # y1 increment: y1D[mh, :, t, :] = UdT[:, :, mh] @ hT
        for mh in (0, 1):
            yp1 = psm.tile([128, 512], F32, tag="sm")
            for kc in (0, 1):
                nc.tensor.matmul(yp1[:, 0:256],
                                 lhsT=UdT_sb[:, kc, mh * 128:(mh + 1) * 128],
                                 rhs=hT[:, kc, :], start=(kc == 0), stop=(kc == 1))
            y1e = bfp.tile([128, 256], BF, tag="y1e")
            nc.vector.tensor_copy(y1e[:], yp1[:, 0:256])
            nc.gpsimd.dma_start(out=y1D[mh, :, t, :], in_=y1e[:])

        if DEBUG and t == 0:
            adma(out=dbg["D_xs0"][:], in_=xs[:])
            de = wpool.tile([128, 256], F32, tag="de")
            nc.vector.tensor_copy(de[:], et[:, 0, :])
            adma(out=dbg["D_e0"][:], in_=de[:])
            adma(out=dbg["D_wx0"][:], in_=expe[:])
            dh = wpool.tile([128, 2, 256], F32, tag="dh")
            nc.vector.tensor_copy(dh[:], h2b[:])
            adma(out=dbg["D_h0"][:], in_=dh[:])
            dy1 = wpool.tile([128, 256], F32, tag="dy1")
            nc.vector.tensor_copy(dy1[:], y1e[:])
            adma(out=dbg["D_y1s"][:], in_=dy1[:])
        if DEBUG and t == T - 1:
            dh = wpool.tile([128, 2, 256], F32, tag="dh")
            nc.vector.tensor_copy(dh[:], h2b[:])
            adma(out=dbg["D_hf"][:], in_=dh[:])
            adma(out=dbg["D_cf"][:], in_=c_st[:])

        prev_hT, prev_cT = hT, cT

    # ---- decoder ----
    prev_dT = zT
    prev_sT = zT
    last_ctT = None
    for t in range(T):
        # x1 = Wd @ [d; s] + Wd_b   -> x1T[m-part(mh), b]
        x1p = psm.tile([128, 512], F32, tag="sm")
        for mh in (0, 1):
            o = x1p[:, mh * 256:(mh + 1) * 256]
            for kc in range(4):
                ds_ = prev_dT if kc < 2 else prev_sT
                nc.tensor.matmul(o, lhsT=WdT_sb[:, kc, mh * 128:(mh + 1) * 128],
                                 rhs=ds_[:, kc % 2, :], start=(kc == 0), stop=False)
            nc.tensor.matmul(o, lhsT=WdTb_sb[0:1, mh * 128:(mh + 1) * 128],
                             rhs=ones1[0:1, 0:256], start=False, stop=True)
        x1b = bfp.tile([128, 2, 256], BF, tag="x1b")
        nc.vector.tensor_copy(x1b[:], x1p[:])

        # gate matmuls that only need prev_dT: emitted early so PE can run
        # them while the attention softmax / bbsel scatter is in flight
        g2 = gps.tile([128, 2, 1024], F32, tag="g")
        for bh in (0, 1):
            sl = slice(bh * 128, (bh + 1) * 128)
            for kc in (0, 1):
                for nx in (0, 1):
                    nc.tensor.matmul(g2[:, bh, nx * 512:(nx + 1) * 512],
                                     lhsT=prev_dT[:, kc, sl],
                                     rhs=decG_sb[:, kc, nx * 512:(nx + 1) * 512],
                                     start=(kc == 0), stop=False)

        # attention: stream y1, add x1 (broadcast over t), tanh, vd-reduce
        l_ps = pAB.tile([32, 512], F32, tag="pA")
        for ci, (tc16, mh) in enumerate([(a, b) for a in range(4) for b in (0, 1)]):
            z8 = ypool.tile([128, 16, 256], BF, tag="ystream")
            sdma(out=z8[:, 0:8, :], in_=y1D[mh, :, tc16 * 16:tc16 * 16 + 8, :])
            adma(out=z8[:, 8:16, :], in_=y1D[mh, :, tc16 * 16 + 8:(tc16 + 1) * 16, :])
            nc.vector.tensor_tensor(z8[:], z8[:],
                                    _bcast(x1b[:, mh, :], 16, 1), OP.add)
            nc.scalar.activation(z8[:], z8[:], AF.Tanh)
            for q in range(8):
                r = tc16 * 8 + q
                nc.tensor.matmul(l_ps[:],
                                 lhsT=vdsel_sb[:, mh, r, :],
                                 rhs=z8[:, 2 * q:2 * q + 2, :],
                                 start=(ci == 0 and q == 0),
                                 stop=(ci == 7 and q == 7))

        # softmax over t (no max-subtract).  l_ps rows r = flat(t*256+b)/512,
        # i.e. row r holds t in {2r, 2r+1} (col block tl) for all b.
        expl = bfp.tile([32, 512], BF, tag="expl")
        nc.scalar.activation(expl[:], l_ps[:], AF.Exp)
        # per-b sums: ones-matmul over the 32 rows, then fold the tl pairs
        sum_ps = pAB.tile([32, 512], F32, tag="pA")
        nc.tensor.matmul(sum_ps[0:1, :], lhsT=ones32[:], rhs=expl[:],
                         start=True, stop=True)
        ssum1 = wpool.tile([1, 256], F32, tag="ssum1")
        sview = bass.AP(tensor=sum_ps[:].tensor, offset=sum_ps[:].offset,
                        ap=[[512, 1], [1, 256], [256, 2]])
        nc.vector.tensor_reduce(ssum1[:], sview, mybir.AxisListType.X, OP.add)
        rs2 = wpool.tile([128, 2, 1], F32, tag="rs2")
        for bh in (0, 1):
            adma(out=rs2[:, bh, :], in_=ssum1[0:1, bh * 128:(bh + 1) * 128])
        nc.vector.reciprocal(rs2[:], rs2[:])
        # scatter UNnormalized exp(l) into the block-diag selector (c_t is
        # normalized at eviction with the f32 1/sum — avoids a systematic
        # bf16 scale error on c_t):
        # bbsel[p'=2c+tl+64par, g, jj, 2jj+par] = exp(l)(b=2(32g+jj)+par, t=2c+tl)
        for par in (0, 1):
            for tl in (0, 1):
                p0 = tl + 64 * par
                for g in range(4):
                    dst = bass.AP(tensor=bbsel[:].tensor,
                                  offset=bbsel[:].offset + p0 * 8192
                                  + g * 2048 + par,
                                  ap=[[2 * 8192, 32], [66, 32]])
                    c0 = tl * 256 + par + 64 * g
                    (sdma if g % 2 == 0 else adma)(
                        out=dst, in_=expl[:, c0: c0 + 63: 2])

        # c_t: selector matmuls, 32-pair accumulation groups
        ctp = pAB.tile([128, 2, 256], F32, tag="pA")
        for i in range(NP):
            g = i >> 5
            jj = i & 31
            bh = i >> 6
            base = 64 * (g & 1)
            nc.tensor.matmul(ctp[base:base + 64, bh, :],
                             lhsT=bbsel[:, g, jj, :], rhs=Et[:, i, :],
                             start=(jj == 0), stop=(jj == 31))
        ctb = bfp.tile([128, 2, 256], BF, tag="ctb")
        nc.vector.tensor_scalar_mul(ctb[:, 0, :], ctp[:, 0, :], rs2[:, 0, 0:1])
        nc.vector.tensor_scalar_mul(ctb[:, 1, :], ctp[:, 1, :], rs2[:, 1, 0:1])

        ctT = tpool.tile([128, 2, 256], BF, tag="ctT")
        for bh in (0, 1):
            for mh in (0, 1):
                tp = psm.tile([128, 128], BF, tag="sm")
                nc.tensor.transpose(tp[:, 0:128],
                                    ctb[:, bh, mh * 128:(mh + 1) * 128], ident[:])
                nc.vector.tensor_copy(ctT[:, mh, bh * 128:(bh + 1) * 128],
                                      tp[:, 0:128])

        # y_til = wt @ c_t (wt_b folded into decG2 bias row)
        ytp = pAB.tile([1, 512], F32, tag="pA")
        for mh in (0, 1):
            nc.tensor.matmul(ytp[:, 0:256], lhsT=wt2_sb[:, mh, :],
                             rhs=ctT[:, mh, :], start=(mh == 0), stop=(mh == 1))
        nc.vector.tensor_copy(ytil1[0:1, :], ytp[:, 0:256])

        # decoder LSTM
        d2b = bfp.tile([128, 2, 256], BF, tag="d2b")
        sbf = bfp.tile([128, 2, 256], BF, tag="sbf")
        final = (t == T - 1)
        if final:
            d2f = wpool.tile([128, 2, 256], F32, tag="d2f")
            ctf = wpool.tile([128, 2, 256], F32, tag="ctf")
            nc.vector.tensor_scalar_mul(ctf[:, 0, :], ctp[:, 0, :], rs2[:, 0, 0:1])
            nc.vector.tensor_scalar_mul(ctf[:, 1, :], ctp[:, 1, :], rs2[:, 1, 0:1])
        for bh in (0, 1):
            sl = slice(bh * 128, (bh + 1) * 128)
            for nx in (0, 1):
                nc.tensor.matmul(g2[:, bh, nx * 512:(nx + 1) * 512],
                                 lhsT=ytil1[0:2, sl],
                                 rhs=decG2_sb[0:2, nx * 512:(nx + 1) * 512],
                                 start=False, stop=True)
        _lstm_pointwise(nc, wpool, g2, s_st[:], d2b[:], sbf[:],
                        h_out_f32=d2f[:] if final else None)

        dT = tpool.tile([128, 2, 256], BF, tag="dT")
        sT = tpool.tile([128, 2, 256], BF, tag="sT")
        for (src, dst) in ((d2b, dT), (sbf, sT)):
            for bh in (0, 1):
                for mh in (0, 1):
                    tp = psm.tile([128, 128], BF, tag="sm")
                    nc.tensor.transpose(tp[:, 0:128],
                                        src[:, bh, mh * 128:(mh + 1) * 128],
                                        ident[:])
                    nc.vector.tensor_copy(dst[:, mh, bh * 128:(bh + 1) * 128],
                                          tp[:, 0:128])

        if DEBUG and t == 0:
            dl = wpool.tile([32, 512], F32, tag="dl")
            nc.vector.tensor_copy(dl[:], expl[:])
            adma(out=dbg["D_expl0"][:], in_=dl[:])
            dct = wpool.tile([128, 2, 256], F32, tag="dct")
            nc.vector.tensor_copy(dct[:], ctb[:])
            adma(out=dbg["D_ct0"][:], in_=dct[:])
            dyt = wpool.tile([1, 256], F32, tag="dyt")
            nc.vector.tensor_copy(dyt[:], ytil1[0:1, :])
            adma(out=dbg["D_yt0"][:], in_=dyt[:])

        prev_dT, prev_sT = dT, sT
        last_ctT = ctT
        if final:
            # f32 transposes of the final d and c_t for the f32 head
            dTf = wpool.tile([128, 2, 256], F32, tag="dTf")
            cTf = wpool.tile([128, 2, 256], F32, tag="cTf")
            for (src, dst) in ((d2f, dTf), (ctf, cTf)):
                for bh in (0, 1):
                    for mh in (0, 1):
                        tp = psm.tile([128, 512], F32, tag="sm")
                        nc.tensor.transpose(tp[:, 0:128],
                                            src[:, bh, mh * 128:(mh + 1) * 128],
                                            identF_sb[:])
                        nc.vector.tensor_copy(dst[:, mh, bh * 128:(bh + 1) * 128],
                                              tp[:, 0:128])
            head_dT, head_cT = dTf, cTf

    # ---- head (f32): out = (dc @ Wy.T + Wy_b) @ vy.T + vy_b ----
    for bh in (0, 1):
        sl = slice(bh * 128, (bh + 1) * 128)
        o1 = psm.tile([128, 512], F32, tag="sm")
        for kc in range(4):
            dc = head_dT if kc < 2 else head_cT
            nc.tensor.matmul(o1[:, 0:256], lhsT=dc[:, kc % 2, sl],
                             rhs=WyT_sb[:, kc, :], start=(kc == 0), stop=False)
        nc.tensor.matmul(o1[:, 0:256], lhsT=ones1f[0:1, 0:128],
                         rhs=WyTb_sb[:], start=False, stop=True)
        tmp = wpool.tile([128, 256], F32, tag="tmp")
        nc.vector.tensor_mul(tmp[:], o1[:, 0:256], vyR_sb[:])
        red = wpool.tile([128, 1], F32, tag="red")
        nc.vector.tensor_reduce(red[:], tmp[:], mybir.AxisListType.X, OP.add)
        ob = wpool.tile([128, 1], F32, tag="ob")
        nc.vector.tensor_scalar_add(ob[:], red[:], vyb_sb[:])
        adma(out=OUT[bh * 128:(bh + 1) * 128, :], in_=ob[:])


def _lstm_pointwise(nc, wpool, g2, c_ap, h_out, c_out_bf, h_out_f32=None):
    """PyTorch LSTMCell pointwise from gate preacts g2 [128, 2(bh), 1024]
    (psum), both b-halves at once via strided [128, 2, .] APs.

    sigmoid(x) = 0.5*tanh(x/2)+0.5.  c_ap [128, 2, 256] updated in place
    (f32); h_out and c_out_bf are bf16 [128, 2, 256] destinations.
    """
    AFt = AF.Tanh
    tif = wpool.tile([128, 2, 512], F32, tag="tif")
    nc.scalar.activation(tif[:], g2[:, :, 0:512], AFt, scale=0.5)
    tg = wpool.tile([128, 2, 256], F32, tag="tg")
    nc.scalar.activation(tg[:], g2[:, :, 512:768], AFt)
    to = wpool.tile([128, 2, 256], F32, tag="to")
    nc.scalar.activation(to[:], g2[:, :, 768:1024], AFt, scale=0.5)
    # sigmoid in place: tif <- (tif+1)*0.5
    nc.vector.tensor_scalar(tif[:], tif[:], 1.0, 0.5, op0=OP.add, op1=OP.mult)
    e1 = wpool.tile([128, 2, 256], F32, tag="e1")
    nc.vector.tensor_mul(e1[:], tif[:, :, 256:512], c_ap)
    nc.vector.tensor_mul(tg[:], tif[:, :, 0:256], tg[:])     # in place
    nc.vector.tensor_add(c_ap, e1[:], tg[:])
    tc2 = wpool.tile([128, 2, 256], F32, tag="tc2")
    nc.scalar.activation(tc2[:], c_ap, AFt)
    nc.vector.tensor_scalar(to[:], to[:], 1.0, 0.5, op0=OP.add, op1=OP.mult)
    nc.vector.tensor_mul(h_out, to[:], tc2[:])
    if h_out_f32 is not None:
        nc.vector.tensor_mul(h_out_f32, to[:], tc2[:])
    nc.vector.tensor_copy(c_out_bf, c_ap)


def _prep(inp):
    """Host-side weight prep (tiny tensors only). Returns per-core common map."""
    f32 = np.float32

    def b(x):
        return np.ascontiguousarray(np.asarray(x, f32).astype(bf16))

    ve = np.asarray(inp["ve_w"], f32)[0]          # [T]
    # vzsel[p', jj, m] = ve[p'%64] iff m == 2*jj + (p'>=64): block-diag
    # selector so 32 pair-matmuls accumulate into distinct row pairs.
    vzsel = np.zeros((128, 32, 64), f32)
    for jj in range(32):
        vzsel[0:64, jj, 2 * jj] = ve
        vzsel[64:128, jj, 2 * jj + 1] = ve
    vd = np.asarray(inp["vd_w"], f32)[0]          # [M]
    vdsel = np.zeros((128, 2, 32, 32), f32)
    for r in range(32):
        vdsel[:, 0, r, r] = vd[0:128]
        vdsel[:, 1, r, r] = vd[128:256]

    com = {
        "UeT": np.ascontiguousarray(np.asarray(inp["Ue_w"], f32).T),
        "vzsel": b(vzsel),
        "vdsel": b(vdsel),
        "WeT": b(np.asarray(inp["We_w"], f32).T),
        "Web128": np.ascontiguousarray(
            np.tile(np.asarray(inp["We_b"], f32), 2)[:, None]),
        "encG": b(np.concatenate([np.asarray(inp["enc_Wih"], f32).T,
                                  np.asarray(inp["enc_Whh"], f32).T], axis=0)),
        "encGb": b((np.asarray(inp["enc_bih"], f32)
                    + np.asarray(inp["enc_bhh"], f32))[None, :]),
        "UdT": b(np.asarray(inp["Ud_w"], f32).T),
        "WdT": b(np.asarray(inp["Wd_w"], f32).T),
        "WdTb": b(np.asarray(inp["Wd_b"], f32)[None, :]),
        "decG": b(np.asarray(inp["dec_Whh"], f32).T),
        "decG2": b(np.stack([
            np.asarray(inp["dec_Wih"], f32)[:, 0],
            np.asarray(inp["dec_bih"], f32) + np.asarray(inp["dec_bhh"], f32)
            + float(np.asarray(inp["wt_b"], f32)[0])
            * np.asarray(inp["dec_Wih"], f32)[:, 0]], axis=0)),
        "wt2": b(np.asarray(inp["wt_w"], f32)[0][:, None]),
        "WyT": np.ascontiguousarray(np.asarray(inp["Wy_w"], f32).T),
        "WyTb": np.ascontiguousarray(np.asarray(inp["Wy_b"], f32)[None, :]),
        "vyR": np.ascontiguousarray(
            np.tile(np.asarray(inp["vy_w"], f32), (128, 1))),
        "vyb128": np.full((128, 1), float(np.asarray(inp["vy_b"], f32)[0]), f32),
        "identD": np.eye(128, dtype=f32).astype(bf16),
        "identF": np.eye(128, dtype=f32),
    }
    return com


def kernel(**inputs):
    global _CACHED_NC
    if _CACHED_NC is None:
        _CACHED_NC = build_nc()
    nc = _CACHED_NC

    com = _prep(inputs)
    Xfull = np.ascontiguousarray(np.asarray(inputs["X_history"], np.float32))
    in_maps = []
    for c in range(NCORES):
        m = dict(com)
        m["X"] = np.ascontiguousarray(Xfull[c * BC:(c + 1) * BC])
        in_maps.append(m)

    trace = bool(int(os.environ.get("DARNN_TRACE", "0")))
    r = run_bass_kernel_spmd(nc, in_maps, list(range(NCORES)), trace=trace)
    res = r.results
    if trace:
        kernel._last_exec_ns = r.exec_time_ns
        kernel._last_profile = r.profile_json
        kernel._trace = r.instructions_and_trace
    out = np.concatenate([res[c]["OUT"] for c in range(NCORES)], axis=0)
    if DEBUG:
        kernel._dbg = res
    return out.astype(np.float32)
